# revision 5
# baseline (speedup 1.0000x reference)
"""FAGCN message-passing layer on 8 Trainium2 NeuronCores (Bass/Tile).

Strategy (dst-partitioned, matmul-scatter):
  - Nodes are 1D-partitioned across the 8 cores (12544 nodes/core, 98
    windows of 128 nodes). Each edge is owned by the core that owns its
    dst node; the host sorts/buckets edges by dst (sharding + index
    layout only - all model compute runs on device).
  - On device, each core builds an augmented gather table
    Haug[n] = [h[n]*d[n] (64), h[n]@W_src + b (1), pad] in fp16 (132B
    rows) plus gd[n] = h[n]@W_dst for its own node slice.
  - Main loop per 128-node window: per-tile indirect DMA gathers fetch
    128 src rows at a time; the gate tanh(gs_src + gd_dst) is computed
    on DVE/ACT with gd_dst expanded from the window's 128 gd values via
    a one-hot (is_equal vs iota) and a multiply-accumulate reduction;
    messages h'*gate are scatter-added into the window's PSUM via
    one-hot matmuls on PE; d_dst is applied at flush.
  - Output z is written per-core as [12544, 64] f32; host concatenates.
"""
import numpy as np

P = 128           # SBUF partitions / window size
D = 64            # feature dim
HC = 66           # Haug row: h' (64) + gs (1) + pad
N_CORES = 8
NPC = 12544       # nodes per core (98 * 128)
NW = NPC // P     # 98 windows per core
R = N_CORES * NPC # padded table rows = 100352
HL_ROWS = 13312   # h_local rows padded to 13*1024 for build batching
BB = 8            # node tiles per build batch


def _host_prep(h, d, gate_W, gate_b, edge_src, edge_dst):
    """Shard + layout preparation (pure data movement / indexing)."""
    N = h.shape[0]

    h_pad = np.zeros((R, D), dtype=np.float32)
    h_pad[:N] = np.asarray(h, dtype=np.float32)
    d_pad = np.zeros((R,), dtype=np.float32)
    d_pad[:N] = np.asarray(d, dtype=np.float32)
    DROWS = d_pad.reshape(R // P, P).T.copy()          # [128, R/128]

    WSRC = np.tile(np.asarray(gate_W[0, D:2 * D], np.float32), (P, 1))  # [128,64]
    WDST = np.tile(np.asarray(gate_W[0, 0:D], np.float32), (P, 1))      # [128,64]
    BREP = np.full((P, 1), float(np.asarray(gate_b).reshape(-1)[0]), np.float32)
    IOTA8 = np.tile(np.arange(P, dtype=np.int8)[None, :], (P, 1))       # [128,128]

    # ---- edge sharding: sort by dst, bucket per core, group by window
    order = np.argsort(edge_dst, kind="stable")
    sd = np.asarray(edge_dst)[order].astype(np.int64)
    ss = np.asarray(edge_src)[order].astype(np.int64)
    bounds = np.searchsorted(sd, np.arange(N_CORES + 1) * NPC)

    per_core = []
    maxcnt = 0
    for c in range(N_CORES):
        lo, hi = int(bounds[c]), int(bounds[c + 1])
        dl = sd[lo:hi] - c * NPC
        src = ss[lo:hi]
        w = dl >> 7
        rel = dl & 127
        counts = np.bincount(w, minlength=NW)
        if counts.size:
            maxcnt = max(maxcnt, int(counts.max()))
        starts = np.zeros(NW, np.int64)
        starts[1:] = np.cumsum(counts)[:-1]
        rank = np.arange(w.size) - starts[w]
        per_core.append((w, src, rel, rank))

    NT = max(1, -(-maxcnt // P))            # tiles per window

    in_maps = []
    for c in range(N_CORES):
        w, src, rel, rank = per_core[c]
        # slot (p, t) of window w holds edge with rank r: p=r%128, t=r//128
        SRCW = np.zeros((NW, P, NT), np.int32)          # pad -> row 0 (killed by drel)
        SRCW[w, rank % P, rank // P] = src.astype(np.int32)
        DREL8 = np.full((NW, P, NT), -1, np.int8)
        DREL8[w, rank % P, rank // P] = rel.astype(np.int8)

        d_local = d_pad[c * NPC:(c + 1) * NPC]
        DCOL = d_local.reshape(NW, P).T.copy()          # [128, NW]
        h_local = np.zeros((HL_ROWS, D), np.float32)
        h_local[:NPC] = h_pad[c * NPC:(c + 1) * NPC]

        in_maps.append({
            "h": h_pad, "drows": DROWS, "wsrc": WSRC, "wdst": WDST,
            "brep": BREP, "iota8": IOTA8,
            "srcw": SRCW.reshape(NW * P, NT),
            "drel": DREL8.reshape(NW * P, NT), "dcol": DCOL, "hloc": h_local,
        })
    return in_maps, NT


def _build_program(NT):
    import concourse.bacc as bacc
    import concourse.tile as tile
    from concourse import bass, mybir

    f32, f16 = mybir.dt.float32, mybir.dt.float16
    i32, i8 = mybir.dt.int32, mybir.dt.int8

    nc = bacc.Bacc("TRN2", target_bir_lowering=False, debug=False,
                   num_devices=N_CORES)
    h_d = nc.dram_tensor("h", [R, D], f32, kind="ExternalInput")
    drows_d = nc.dram_tensor("drows", [P, R // P], f32, kind="ExternalInput")
    wsrc_d = nc.dram_tensor("wsrc", [P, D], f32, kind="ExternalInput")
    wdst_d = nc.dram_tensor("wdst", [P, D], f32, kind="ExternalInput")
    brep_d = nc.dram_tensor("brep", [P, 1], f32, kind="ExternalInput")
    iota_d = nc.dram_tensor("iota8", [P, P], i8, kind="ExternalInput")
    srcw_d = nc.dram_tensor("srcw", [NW * P, NT], i32, kind="ExternalInput")
    drel_d = nc.dram_tensor("drel", [NW * P, NT], i8, kind="ExternalInput")
    dcol_d = nc.dram_tensor("dcol", [P, NW], f32, kind="ExternalInput")
    hloc_d = nc.dram_tensor("hloc", [HL_ROWS, D], f32, kind="ExternalInput")
    z_d = nc.dram_tensor("z", [NPC, D], f32, kind="ExternalOutput")

    haug_i = nc.dram_tensor("haug_i", [R, HC], f16, kind="Internal")
    gdl_i = nc.dram_tensor("gdl_i", [HL_ROWS], f16, kind="Internal")

    with tile.TileContext(nc) as tc:
        with (
            tc.tile_pool(name="const", bufs=1) as cp,
            tc.tile_pool(name="bld", bufs=2) as bp,
            tc.tile_pool(name="main", bufs=2) as mp,
            tc.tile_pool(name="psum", bufs=2, space="PSUM") as pp,
        ):
            wsrc_t = cp.tile([P, D], f32)
            nc.sync.dma_start(out=wsrc_t[:], in_=wsrc_d[:, :])
            wdst_t = cp.tile([P, D], f32)
            nc.sync.dma_start(out=wdst_t[:], in_=wdst_d[:, :])
            brep_t = cp.tile([P, 1], f32)
            nc.sync.dma_start(out=brep_t[:], in_=brep_d[:, :])
            iota_t = cp.tile([P, P], i8)
            nc.sync.dma_start(out=iota_t[:], in_=iota_d[:, :])
            dcol_t = cp.tile([P, NW], f32)
            nc.sync.dma_start(out=dcol_t[:], in_=dcol_d[:, :])

            # ---------- build phase: Haug (all nodes) ----------
            for s in range(R // (BB * P)):
                rows = slice(s * BB * P, (s + 1) * BB * P)
                h8 = bp.tile([P, BB, D], f32, tag="h8")
                nc.sync.dma_start(
                    out=h8[:],
                    in_=h_d[rows, :].rearrange("(j p) e -> p j e", p=P))
                dd8 = bp.tile([P, BB], f32, tag="dd8")
                nc.sync.dma_start(out=dd8[:], in_=drows_d[:, s * BB:(s + 1) * BB])
                hp16 = bp.tile([P, BB, HC], f16, tag="hp16")
                nc.vector.tensor_tensor(
                    out=hp16[:, :, 0:D], in0=h8[:],
                    in1=dd8[:].rearrange("p (j a) -> p j a", a=1).to_broadcast([P, BB, D]),
                    op=mybir.AluOpType.mult)
                gsb = bp.tile([P, BB], f32, tag="gsb")
                sc64 = bp.tile([P, D], f32, tag="sc64")
                for j in range(BB):
                    nc.vector.scalar_tensor_tensor(
                        out=sc64[:], in0=h8[:, j, :], scalar=1.0, in1=wsrc_t[:],
                        op0=mybir.AluOpType.mult, op1=mybir.AluOpType.mult,
                        accum_out=gsb[:, j:j + 1])
                nc.vector.tensor_scalar(
                    out=hp16[:, :, D], in0=gsb[:], scalar1=brep_t[:, 0:1],
                    scalar2=None, op0=mybir.AluOpType.add)
                nc.sync.dma_start(
                    out=haug_i[rows, :].rearrange("(j p) e -> p j e", p=P),
                    in_=hp16[:])

            # ---------- build phase: gd for local nodes ----------
            for s in range(HL_ROWS // (BB * P)):
                rows = slice(s * BB * P, (s + 1) * BB * P)
                hl8 = bp.tile([P, BB, D], f32, tag="h8")
                nc.sync.dma_start(
                    out=hl8[:],
                    in_=hloc_d[rows, :].rearrange("(j p) e -> p j e", p=P))
                gdb = bp.tile([P, BB], f32, tag="gsb")
                sc64b = bp.tile([P, D], f32, tag="sc64")
                for j in range(BB):
                    nc.vector.scalar_tensor_tensor(
                        out=sc64b[:], in0=hl8[:, j, :], scalar=1.0, in1=wdst_t[:],
                        op0=mybir.AluOpType.mult, op1=mybir.AluOpType.mult,
                        accum_out=gdb[:, j:j + 1])
                gd16 = bp.tile([P, BB], f16, tag="gd16")
                nc.vector.tensor_copy(out=gd16[:], in_=gdb[:])
                with nc.allow_non_contiguous_dma(reason="tiny gdl transpose write"):
                    nc.sync.dma_start(
                        out=gdl_i.ap()[rows].rearrange("(j p) -> p j", p=P),
                        in_=gd16[:])

            # ---------- main loop: one 128-node window at a time ----------
            for w in range(NW):
                srcw_t = mp.tile([P, NT], i32, tag="srcw")
                nc.sync.dma_start(out=srcw_t[:], in_=srcw_d[w * P:(w + 1) * P, :])
                drel_t = mp.tile([P, NT], i8, tag="drel")
                nc.sync.dma_start(out=drel_t[:], in_=drel_d[w * P:(w + 1) * P, :])
                gdrep_t = mp.tile([P, P], f16, tag="gdrep")
                nc.sync.dma_start(
                    out=gdrep_t[:],
                    in_=gdl_i.ap()[w * P:(w + 1) * P].partition_broadcast(P))

                ga = mp.tile([P, NT, HC], f16, tag="ga")
                for t in range(NT):
                    nc.gpsimd.indirect_dma_start(
                        out=ga[:, t, :], out_offset=None,
                        in_=haug_i[:, :],
                        in_offset=bass.IndirectOffsetOnAxis(
                            ap=srcw_t[:, t:t + 1], axis=0))

                onehot = mp.tile([P, NT, P], f16, tag="onehot")
                nc.vector.tensor_tensor(
                    out=onehot[:],
                    in0=iota_t[:].rearrange("p (a e) -> p a e", a=1).to_broadcast([P, NT, P]),
                    in1=drel_t[:].rearrange("p (t a) -> p t a", a=1).to_broadcast([P, NT, P]),
                    op=mybir.AluOpType.is_equal)

                gdd = mp.tile([P, NT], f32, tag="gdd")
                scr = mp.tile([P, P], f16, tag="scr")
                for t in range(NT):
                    nc.vector.scalar_tensor_tensor(
                        out=scr[:], in0=onehot[:, t, :], scalar=1.0, in1=gdrep_t[:],
                        op0=mybir.AluOpType.mult, op1=mybir.AluOpType.mult,
                        accum_out=gdd[:, t:t + 1])

                pre = mp.tile([P, NT], f32, tag="pre")
                nc.vector.tensor_tensor(out=pre[:], in0=ga[:, :, D], in1=gdd[:],
                                        op=mybir.AluOpType.add)
                th = mp.tile([P, NT], f32, tag="th")
                nc.scalar.activation(out=th[:], in_=pre[:],
                                     func=mybir.ActivationFunctionType.Tanh)
                msg = mp.tile([P, NT, D], f16, tag="msg")
                nc.vector.tensor_tensor(
                    out=msg[:], in0=ga[:, :, 0:D],
                    in1=th[:].rearrange("p (t a) -> p t a", a=1).to_broadcast([P, NT, D]),
                    op=mybir.AluOpType.mult)

                ps = pp.tile([P, D], f32, tag="ps")
                for t in range(NT):
                    nc.tensor.matmul(out=ps[:], lhsT=onehot[:, t, :],
                                     rhs=msg[:, t, :],
                                     start=(t == 0), stop=(t == NT - 1))
                zs = mp.tile([P, D], f32, tag="zs")
                nc.vector.tensor_scalar_mul(zs[:], ps[:], dcol_t[:, w:w + 1])
                nc.sync.dma_start(out=z_d[w * P:(w + 1) * P, :], in_=zs[:])

    nc.compile()
    return nc


_CACHE = {}


def kernel(h, d, gate_W, gate_b, edge_src, edge_dst):
    from concourse.bass_utils import run_bass_kernel_spmd

    N = h.shape[0]
    in_maps, NT = _host_prep(h, d, gate_W, gate_b, edge_src, edge_dst)
    if NT not in _CACHE:
        _CACHE[NT] = _build_program(NT)
    nc = _CACHE[NT]
    res = run_bass_kernel_spmd(nc, in_maps, core_ids=list(range(N_CORES)))
    z = np.concatenate([res.results[c]["z"] for c in range(N_CORES)], axis=0)
    return np.ascontiguousarray(z[:N]).astype(np.float32)
